# revision 1
# baseline (speedup 1.0000x reference)
"""Trainium2 Bass kernel: weighted-KDE avoid-distance (retrieval_knn).

dist[n] = mean_m exp(-0.5 * sum_d (means[m,d]-samples[n,d])^2 / stds[m,d])
out     = -dist + max(dist) + min(dist)

Strategy: data-parallel over the N=8192 samples axis across 8 NeuronCores
(1024 samples each; every core holds the full means/stds buffer).

Per-core math is reformulated as one K=256 matmul + fused exp-accumulate:
  logp[n,m] = sB.mB + s2.w' + a[m]
    w'  = -0.5/std,  sB = -2*s,  mB = m*w',  s2 = s*s,  a[m] = sum_d m^2*w'
All operands are split hi/lo in bf16 (hi = bf16(x), lo = bf16(x - hi)) so the
TensorE bf16 matmul reproduces fp32-level accuracy (~2^-18 per operand):
  pass1 (K=128): [sB_hi sB_lo s2_hi s2_lo] x [mB_hi mB_hi w'_hi w'_hi]
  pass2 (K=128): [sB_hi s2_hi ones(64)]    x [mB_lo w'_lo  mq_hi mq_lo]
with mq = m^2*w' (the a[m] term, summed by the matmul itself via ones-rows).
ScalarE does one exp per 128-sample chunk over the whole [128, 2048] PSUM
tile with a fused per-partition accumulate over the free (m) axis; bias
-ln(2048) folds the mean's 1/M into the exponent.

Feature-major operands are produced by computing features in natural layout
and packing them as bf16 columns. The pass-1 operand bounces through DRAM
and returns via the 2-byte xbar DMA-transpose; the sample and pass-2
operands transpose on the TensorE against a bf16 identity (batched four
tiles per PSUM bank with one wide VectorE copy out). Splitting the two rhs
operands across the DMA and PE transpose paths lets them proceed in
parallel, and the back-to-back PE transposes double as the tensor engine's
p-state warmup, so the main matmuls start at full clock. Both paths emit
the same column order (col t*128+q <-> m = q*MT+t, via t-major staging
rows for the DMA path), which is required since both passes accumulate
into the same PSUM columns; the m permutation itself is harmless because
m is fully reduced.

The final flip (-dist + max + min) is a trivial O(N) op done on host after
gathering the 8 shards.
"""

import sys

import numpy as np

for _p in ("/opt/trn_rl_repo", "/root/.axon_site/_ro/trn_rl_repo"):
    if _p not in sys.path:
        sys.path.insert(0, _p)

N, M, D = 8192, 2048, 32
N_CORES = 8
NSH = N // N_CORES        # 1024 samples per core
MT = M // 128             # 16 mean tiles
CT = NSH // 128           # 8 sample chunks per core
MJ = M // 512             # 4 matmul moving slices
LN_M = float(np.log(M))   # ln(2048); exp bias folds the 1/M mean

_CACHE = {}


def _build_nc(reps: int = 1):
    # reps>1 repeats the whole compute body inside one NEFF (used only by
    # test.py to measure per-iteration HW time by wall-clock delta).
    import concourse.bacc as bacc
    import concourse.tile as tile
    from concourse import mybir
    from concourse.masks import make_identity

    f32 = mybir.dt.float32
    bf16 = mybir.dt.bfloat16
    AF = mybir.ActivationFunctionType
    OP = mybir.AluOpType

    nc = bacc.Bacc("TRN2", target_bir_lowering=False, debug=False)

    samples_d = nc.dram_tensor("samples", [NSH, D], f32, kind="ExternalInput")
    means_d = nc.dram_tensor("means", [M, D], f32, kind="ExternalInput")
    stds_d = nc.dram_tensor("stds", [M, D], f32, kind="ExternalInput")
    dist_d = nc.dram_tensor("dist", [NSH], f32, kind="ExternalOutput")
    # DRAM bounce buffers for the 2-byte xbar transposes (mean side)
    stg1_d = nc.dram_tensor("stg1", [M, 128], bf16)

    with tile.TileContext(nc) as tc:
        with (
            tc.tile_pool(name="persist", bufs=1) as pp,
            tc.tile_pool(name="psum", bufs=2, space="PSUM") as psp,
            tc.tile_pool(name="expo", bufs=4) as xp,
        ):
          for _rep in range(reps):
            # ---- load inputs, contiguous per partition ----
            # Layout [p, t, d] with m = p*MT + t (and n = p*CT + c): one 2KB
            # descriptor per partition, and the transposed column index below
            # comes out as exactly m (resp. n).
            samp_nat = pp.tile([128, CT, D], f32)
            nc.sync.dma_start(samp_nat[:], samples_d.ap().rearrange("(p c) d -> p c d", p=128))
            means_nat = pp.tile([128, MT, D], f32)
            stds_nat = pp.tile([128, MT, D], f32)
            nc.scalar.dma_start(stds_nat[:], stds_d.ap().rearrange("(p t) d -> p t d", p=128))
            nc.scalar.dma_start(means_nat[:], means_d.ap().rearrange("(p t) d -> p t d", p=128))

            # bf16 identity for PE-based transposes (sample side)
            identity = pp.tile([128, 128], bf16)
            make_identity(nc, identity[:])

            # ---- mean-side features, natural layout ----
            # DVE order is deliberate: the pass-1 (hi) features come first so
            # the pass-1 store/transpose chain can start while the lo features
            # are still being computed.
            r = pp.tile([128, MT, D], f32)       # 1/std
            mB = pp.tile([128, MT, D], f32)      # m * w' = -0.5*m/std
            t2 = pp.tile([128, MT, D], f32)      # m^2 * w'
            nc.vector.reciprocal(r[:], stds_nat[:])
            nc.vector.scalar_tensor_tensor(
                mB[:], means_nat[:], -0.5, r[:], op0=OP.mult, op1=OP.mult)

            packed1 = pp.tile([128, MT, 128], bf16)
            packed2 = pp.tile([128, MT, 128], bf16)
            # pass1 mean cols: [mB_hi, mB_hi, w'_hi, w'_hi] — dups written by
            # DVE right after each source so the staging store isn't gated on
            # the Pool queue
            nc.vector.tensor_scalar_mul(packed1[:, :, 2 * D:3 * D], r[:], -0.5)  # w'_hi
            nc.vector.tensor_scalar_mul(packed1[:, :, 3 * D:4 * D], r[:], -0.5)  # dup
            nc.vector.tensor_copy(packed1[:, :, 0:D], mB[:])                    # mB_hi
            nc.vector.tensor_copy(packed1[:, :, D:2 * D], mB[:])                # dup
            # pass2 mean cols: [mB_lo, w'_lo, mq_hi, mq_lo]
            nc.vector.scalar_tensor_tensor(                                     # mB - mB_hi
                packed2[:, :, 0:D], mB[:], 1.0, packed1[:, :, 0:D],
                op0=OP.mult, op1=OP.subtract)
            nc.vector.scalar_tensor_tensor(                                     # -0.5r - w'_hi
                packed2[:, :, D:2 * D], r[:], -0.5, packed1[:, :, 2 * D:3 * D],
                op0=OP.mult, op1=OP.subtract)
            nc.vector.tensor_mul(t2[:], means_nat[:], mB[:])
            nc.vector.tensor_copy(packed2[:, :, 2 * D:3 * D], t2[:])            # mq_hi
            nc.vector.scalar_tensor_tensor(                                     # mq - mq_hi
                packed2[:, :, 3 * D:4 * D], t2[:], 1.0, packed2[:, :, 2 * D:3 * D],
                op0=OP.mult, op1=OP.subtract)

            # ---- sample-side features + PE transpose (no DRAM bounce) ----
            s2 = pp.tile([128, CT, D], f32)
            nc.vector.tensor_mul(s2[:], samp_nat[:], samp_nat[:])
            spacked = pp.tile([128, CT, 128], bf16)  # [sB_hi, sB_lo, s2_hi, s2_lo]
            nc.vector.tensor_scalar_mul(spacked[:, :, 0:D], samp_nat[:], -2.0)  # sB_hi
            nc.vector.scalar_tensor_tensor(
                spacked[:, :, D:2 * D], samp_nat[:], -2.0, spacked[:, :, 0:D],
                op0=OP.mult, op1=OP.subtract)                                   # sB_lo
            nc.vector.tensor_copy(spacked[:, :, 2 * D:3 * D], s2[:])            # s2_hi
            nc.vector.scalar_tensor_tensor(
                spacked[:, :, 3 * D:4 * D], s2[:], 1.0, spacked[:, :, 2 * D:3 * D],
                op0=OP.mult, op1=OP.subtract)                                   # s2_lo
            s1T = pp.tile([128, NSH], bf16)   # col n: [sB_hi, sB_lo, s2_hi, s2_lo]
            for c in range(CT):
                tp = psp.tile([128, 128], bf16, tag="ps")
                nc.tensor.transpose(tp[:], spacked[:, c, :], identity[:])
                nc.vector.tensor_copy(s1T[:, c * 128:(c + 1) * 128], tp[:])
            # pass2 sample rows: [sB_hi, s2_hi, ones(64)] (ones pair with mq rows)
            s2T = pp.tile([128, NSH], bf16)
            nc.gpsimd.memset(s2T[2 * D:4 * D, :], 1.0)
            nc.vector.tensor_copy(s2T[0:D, :], s1T[0:D, :])
            nc.vector.tensor_copy(s2T[D:2 * D, :], s1T[2 * D:3 * D, :])

            # ---- pass1 via DRAM bounce + xbar transpose; pass2 via PE
            # transposes (keeps the PE warm and halves the SP queue work).
            # rhs1 staging rows are t-major (r = t*128+p) so both paths
            # produce the same column order: col t*128+x <-> m = x*MT+t.
            rhs1 = pp.tile([128, M], bf16)
            rhs2 = pp.tile([128, M], bf16)
            stg1v = stg1_d.ap().rearrange("(t p) f -> p t f", p=128)
            # batched stores then transposes: each DMACopy <-> DmaTransposeAnt
            # xbar-mode switch serializes the queue (~3us)
            for h in range(2):
                ts_ = slice(h * (MT // 2), (h + 1) * (MT // 2))
                nc.sync.dma_start(stg1v[:, ts_, :], packed1[:, ts_, :])
            for h in range(2):
                rg = slice(h * 1024, (h + 1) * 1024)
                nc.sync.dma_start(rhs1[:, rg], stg1_d.ap()[rg, :], transpose=True)
            for b in range(4):
                tpb = psp.tile([128, 4, 128], bf16, tag="ps")
                for k in range(4):
                    nc.tensor.transpose(tpb[:, k, :], packed2[:, 4 * b + k, :], identity[:])
                nc.vector.tensor_copy(
                    rhs2[:, b * 512:(b + 1) * 512],
                    tpb[:].rearrange("p a f -> p (a f)"))

            # ---- main loop: 8 matmuls + one exp-accumulate per chunk ----
            ebias = pp.tile([128, 1], f32)   # exp bias: -ln(M) folds the 1/M mean
            nc.gpsimd.memset(ebias[:], -LN_M)
            dist_sb = pp.tile([128, CT], f32)
            for c in range(CT):
                ps = psp.tile([128, M], f32)  # 4 PSUM banks
                lhs1 = s1T[:, c * 128:(c + 1) * 128]
                lhs2 = s2T[:, c * 128:(c + 1) * 128]
                # all pass-1 slices share lhs1, then all pass-2 share lhs2:
                # one stationary load per pass instead of one per matmul
                for j in range(MJ):
                    sl = slice(j * 512, (j + 1) * 512)
                    nc.tensor.matmul(ps[:, sl], lhsT=lhs1, rhs=rhs1[:, sl],
                                     start=True, stop=False, skip_group_check=True)
                for j in range(MJ):
                    sl = slice(j * 512, (j + 1) * 512)
                    nc.tensor.matmul(ps[:, sl], lhsT=lhs2, rhs=rhs2[:, sl],
                                     start=False, stop=True, skip_group_check=True)
                eo = xp.tile([128, M], bf16)
                nc.scalar.activation(eo[:], ps[:], AF.Exp, bias=ebias[:],
                                     scale=1.0, accum_out=dist_sb[:, c:c + 1])

            # psum partition q of chunk c is n = q*CT + c, so the "(p c)" view
            # writes dist in natural order
            nc.sync.dma_start(dist_d.ap().rearrange("(p c) -> p c", p=128), dist_sb[:])

    nc.compile()
    return nc


def _get_nc():
    if "nc" not in _CACHE:
        _CACHE["nc"] = _build_nc()
    return _CACHE["nc"]


def kernel(samples: np.ndarray, means: np.ndarray, stds: np.ndarray) -> np.ndarray:
    from concourse.bass_utils import run_bass_kernel_spmd

    samples = np.ascontiguousarray(samples, dtype=np.float32)
    means = np.ascontiguousarray(means, dtype=np.float32)
    stds = np.ascontiguousarray(stds, dtype=np.float32)

    nc = _get_nc()
    in_maps = [
        {"samples": samples[i * NSH:(i + 1) * NSH], "means": means, "stds": stds}
        for i in range(N_CORES)
    ]
    res = run_bass_kernel_spmd(nc, in_maps, list(range(N_CORES)))
    dist = np.concatenate([res.results[i]["dist"] for i in range(N_CORES)])
    return (-dist + dist.max() + dist.min()).astype(np.float32)



# revision 29
# speedup vs baseline: 610.6574x; 610.6574x over previous
"""Trainium2 Bass kernel: weighted-KDE avoid-distance (retrieval_knn).

dist[n] = mean_m exp(-0.5 * sum_d (means[m,d]-samples[n,d])^2 / stds[m,d])
out     = -dist + max(dist) + min(dist)

Strategy: data-parallel over the N=8192 samples axis across 8 NeuronCores
(1024 samples each; every core holds the full means/stds buffer).

Per-core math is TWO K=128 fp16 matmul passes + fused exp-accumulate, with
every operand split hi/lo in fp16 (hi = fp16(x), lo = fp16(x - hi)) so the
TensorE reproduces fp32-level accuracy (~2^-23 per operand; HW error was
measured 4x the simulated estimate for single-fp16 operands, which busted
the 2e-2 tolerance — hi/lo splits make that margin moot):
  logp[n,m] = sB.mB + s2.w + sum_d (t2_hi + t2_lo),
    sB = -2*s, s2 = s*s, mB = -0.5*m/std, w = -0.5/std, t2 = m*mB
  pass1: lhsT rows [sB_hi; s2_hi; sB_lo; s2_lo] x rhs rows [mB_hi; w_hi;
         mB_hi; w_hi] — the rhs tile is two PE transposes of the SAME
         64-column [mB_hi | w_hi] block into the upper/lower partition
         halves, so no duplicated packing writes are needed
  pass2: lhsT rows [sB_hi; s2_hi; ones(64)] x rhs rows [mB_lo; w_lo;
         t2_hi; t2_lo] — the ones rows make the matmul itself reduce the
         sample-independent a[m] term at fp32 precision
Both passes accumulate into the same PSUM columns (identical per-tile
column order); m is fully reduced so the m-permutation is harmless.

ScalarE is the floor: one exp over each [128, 2048] PSUM chunk with fused
accumulate = 8 x ~2.1us; bias -ln(2048) folds the mean's 1/M into the
exponent.  TensorE (64 matmuls + transposes ~14us) and DVE (feature
prep/copies ~12us) stay under that umbrella.  Latency details:
  - dummy exp at t~0 pulls the ~2.7us ACT table load off the critical path
  - one dma_start per input (each pays ~0.6us serialized HWDGE dispatch +
    ~0.9us semaphore latency), ordered stds, samples, means
  - ~20 dummy identity-transposes warm the PE (HAM 1.2 -> 2.4 GHz) during
    the DMA window
  - PSUM tensor allocation order (sample-T, pass1-T, pass2-T, then matmul
    chunks) keeps the two 4-bank PSUM slots free of circular waits
  - chunk 0's exp is split in two 1024-col halves (extra accumulator
    column merged by one DVE add) so ScalarE starts before all of rhs
    exists (reps=1 only; steady-state reps use the unsplit form)

The final flip (-dist + max + min) is a trivial O(N) op done on host after
gathering the 8 shards.
"""

import sys

import numpy as np

for _p in ("/opt/trn_rl_repo", "/root/.axon_site/_ro/trn_rl_repo"):
    if _p not in sys.path:
        sys.path.insert(0, _p)

N, M, D = 8192, 2048, 32
N_CORES = 8
NSH = N // N_CORES        # 1024 samples per core
MT = M // 128             # 16 mean tiles
CT = NSH // 128           # 8 sample chunks per core
MJ = M // 512             # 4 matmul moving slices
LN_M = float(np.log(M))   # ln(2048); exp bias folds the 1/M mean

_CACHE = {}


def _build_nc(reps: int = 1):
    # reps>1 repeats the whole compute body inside one NEFF (used only by
    # test.py to measure per-iteration HW time by wall-clock delta).
    import concourse.bacc as bacc
    import concourse.tile as tile
    from concourse import mybir
    from concourse.masks import make_identity

    f32 = mybir.dt.float32
    f16 = mybir.dt.float16
    bf16 = mybir.dt.bfloat16
    AF = mybir.ActivationFunctionType
    OP = mybir.AluOpType

    nc = bacc.Bacc("TRN2", target_bir_lowering=False, debug=False)

    samples_d = nc.dram_tensor("samples", [NSH, D], f32, kind="ExternalInput")
    means_d = nc.dram_tensor("means", [M, D], f32, kind="ExternalInput")
    stds_d = nc.dram_tensor("stds", [M, D], f32, kind="ExternalInput")
    dist_d = nc.dram_tensor("dist", [NSH], f32, kind="ExternalOutput")

    HT = MT // 2  # t-tiles per half (feature-op granularity)

    with tile.TileContext(nc) as tc:
        with (
            tc.tile_pool(name="persist", bufs=2) as pp,
            tc.tile_pool(name="psum", bufs=2, space="PSUM") as psp,
            tc.tile_pool(name="expo", bufs=2) as xp,
        ):
          for _rep in range(reps):
            split0 = _rep == 0  # split chunk 0's exp only in the first rep

            # ---- input loads (one DMA each; HWDGE dispatch + the global
            # transfer engine serialize in emission order) ----
            stds_nat = pp.tile([128, MT, D], f32)
            means_nat = pp.tile([128, MT, D], f32)
            samp_nat = pp.tile([128, CT, D], f32)
            nc.sync.dma_start(stds_nat[:], stds_d.ap().rearrange("(p t) d -> p t d", p=128))
            nc.scalar.dma_start(samp_nat[:], samples_d.ap().rearrange("(p c) d -> p c d", p=128))
            nc.sync.dma_start(means_nat[:], means_d.ap().rearrange("(p t) d -> p t d", p=128))

            # exp bias (-ln M) + dummy exp: ACT table load at t~0
            ebias = pp.tile([128, 1], f32)
            nc.gpsimd.memset(ebias[:], -LN_M)
            warm = pp.tile([128, 1], f32)
            nc.scalar.activation(warm[:], ebias[:], AF.Exp)

            # fp16 identity for PE-based transposes
            identity = pp.tile([128, 128], f16)
            make_identity(nc, identity[:])

            # sample-side features: cols [sB_hi | s2_hi | sB_lo | s2_lo]
            # (this column order becomes the lhsT row order pass1 needs)
            spacked = pp.tile([128, CT, 128], f16)
            s2f = pp.tile([128, CT, D], f32)

            # mean-side: packed1 cols [mB_hi | w_hi], packed2 cols
            # [mB_lo | w_lo | t2_hi | t2_lo]
            r = pp.tile([128, MT, D], f32)
            mBf = pp.tile([128, MT, D], f32)
            t2f = pp.tile([128, MT, D], f32)
            packed1 = pp.tile([128, MT, 2 * D], f16)
            packed2 = pp.tile([128, MT, 128], f16)

            s1T = pp.tile([128, NSH], f16)
            s2T = pp.tile([128, NSH], f16)
            nc.gpsimd.memset(s2T[2 * D:4 * D, :], 1.0)
            rhs1 = pp.tile([128, M], f16)
            rhs2 = pp.tile([128, M], f16)
            dist_sb = pp.tile([128, CT], f32)
            parts = pp.tile([128, 2], f32)

            hs_ = [slice(h * HT, (h + 1) * HT) for h in range(2)]

            # ---- DVE feature chain, in DMA-arrival order (strict FIFO):
            # stds -> recips; samples -> full sample block; means -> pass1
            # features (mB_hi) for both halves, THEN all pass2 features.
            nc.vector.reciprocal(r[:, hs_[0]], stds_nat[:, hs_[0]])
            nc.vector.tensor_scalar_mul(packed1[:, hs_[0], D:2 * D], r[:, hs_[0]], -0.5)  # w_hi
            nc.vector.tensor_scalar_mul(spacked[:, :, 0:D], samp_nat[:], -2.0)            # sB_hi
            nc.vector.tensor_mul(s2f[:], samp_nat[:], samp_nat[:])
            nc.vector.tensor_copy(spacked[:, :, D:2 * D], s2f[:])                         # s2_hi
            nc.vector.scalar_tensor_tensor(
                spacked[:, :, 2 * D:3 * D], samp_nat[:], -2.0, spacked[:, :, 0:D],
                op0=OP.mult, op1=OP.subtract)                                             # sB_lo
            nc.vector.scalar_tensor_tensor(
                spacked[:, :, 3 * D:4 * D], s2f[:], 1.0, spacked[:, :, D:2 * D],
                op0=OP.mult, op1=OP.subtract)                                             # s2_lo
            nc.vector.reciprocal(r[:, hs_[1]], stds_nat[:, hs_[1]])
            nc.vector.tensor_scalar_mul(packed1[:, hs_[1], D:2 * D], r[:, hs_[1]], -0.5)  # w_hi
            for h in range(2):
                hs = hs_[h]
                nc.vector.scalar_tensor_tensor(
                    mBf[:, hs], means_nat[:, hs], -0.5, r[:, hs],
                    op0=OP.mult, op1=OP.mult)
                nc.vector.tensor_copy(packed1[:, hs, 0:D], mBf[:, hs])                    # mB_hi
            for h in range(2):
                hs = hs_[h]
                nc.vector.tensor_mul(t2f[:, hs], means_nat[:, hs], mBf[:, hs])
                nc.vector.scalar_tensor_tensor(
                    packed2[:, hs, 0:D], mBf[:, hs], 1.0, packed1[:, hs, 0:D],
                    op0=OP.mult, op1=OP.subtract)                                         # mB_lo
                nc.vector.scalar_tensor_tensor(
                    packed2[:, hs, D:2 * D], r[:, hs], -0.5, packed1[:, hs, D:2 * D],
                    op0=OP.mult, op1=OP.subtract)                                         # w_lo
                nc.vector.tensor_copy(packed2[:, hs, 2 * D:3 * D], t2f[:, hs])            # t2_hi
                nc.vector.scalar_tensor_tensor(
                    packed2[:, hs, 3 * D:4 * D], t2f[:, hs], 1.0, packed2[:, hs, 2 * D:3 * D],
                    op0=OP.mult, op1=OP.subtract)                                         # t2_lo

            # ---- PE transposes.  PSUM tensor alloc order: sample-T (A),
            # pass1-T (B), pass2-T (A), mm chunks (B, A, ...) — no slot
            # cycle ever waits on a later producer.
            tps = psp.tile([128, CT, 128], f16, tag="ps")
            if split0:  # PE warmup only needed from cold start
                for _ in range(20):
                    nc.tensor.transpose(tps[:, 0, :], identity[:], identity[:])
            for c in range(CT):
                nc.tensor.transpose(tps[:, c, :], spacked[:, c, :], identity[:])
            # rep 0: PSUM->SBUF copies ride the idle ScalarE (latency);
            # later reps: DVE, so the saturated exp stream isn't extended.
            cp = nc.scalar.copy if split0 else nc.vector.tensor_copy
            cp(s1T[:], tps[:].rearrange("p a f -> p (a f)"))
            # pass2 lhsT rows [sB_hi; s2_hi] are s1T rows 0:64 (ones below)
            cp(s2T[0:2 * D, :], s1T[0:2 * D, :])

            tpm1 = psp.tile([128, MT, 128], f16, tag="ps")
            for h in range(2):
                for k in range(HT):
                    t = h * HT + k
                    # same 64-col [mB_hi | w_hi] block into both partition
                    # halves: rows become [mB_hi; w_hi; mB_hi; w_hi]
                    nc.tensor.transpose(tpm1[0:2 * D, t, :], packed1[:, t, :], identity[:])
                    nc.tensor.transpose(tpm1[2 * D:4 * D, t, :], packed1[:, t, :], identity[:])
                cp(rhs1[:, h * 1024:(h + 1) * 1024],
                   tpm1[:, hs_[h]].rearrange("p a f -> p (a f)"))

            tpm2 = psp.tile([128, MT, 128], f16, tag="ps")
            for h in range(2):
                for k in range(HT):
                    t = h * HT + k
                    nc.tensor.transpose(tpm2[:, t, :], packed2[:, t, :], identity[:])
                nc.vector.tensor_copy(
                    rhs2[:, h * 1024:(h + 1) * 1024],
                    tpm2[:, hs_[h]].rearrange("p a f -> p (a f)"))

            # ---- main loop: 8 matmuls (2 passes) + exp per chunk ----
            def chunk_mms(ps, c, j0, j1):
                lhs1 = s1T[:, c * 128:(c + 1) * 128]
                lhs2 = s2T[:, c * 128:(c + 1) * 128]
                for j in range(j0, j1):
                    sl = slice(j * 512, (j + 1) * 512)
                    nc.tensor.matmul(ps[:, sl], lhsT=lhs1, rhs=rhs1[:, sl],
                                     start=True, stop=False, skip_group_check=True)
                for j in range(j0, j1):
                    sl = slice(j * 512, (j + 1) * 512)
                    nc.tensor.matmul(ps[:, sl], lhsT=lhs2, rhs=rhs2[:, sl],
                                     start=False, stop=True, skip_group_check=True)

            # chunk 0 (exp split in two 1024-col halves in the first rep)
            ps = psp.tile([128, M], f32, tag="ps")
            eo = xp.tile([128, M], bf16)
            if split0:
                for half in range(2):
                    sl = slice(half * 1024, (half + 1) * 1024)
                    chunk_mms(ps, 0, 2 * half, 2 * half + 2)
                    nc.scalar.activation(eo[:, sl], ps[:, sl], AF.Exp,
                                         bias=ebias[:], scale=1.0,
                                         accum_out=parts[:, half:half + 1])
                nc.vector.tensor_add(dist_sb[:, 0:1], parts[:, 0:1], parts[:, 1:2])
            else:
                chunk_mms(ps, 0, 0, MJ)
                nc.scalar.activation(eo[:], ps[:], AF.Exp, bias=ebias[:],
                                     scale=1.0, accum_out=dist_sb[:, 0:1])

            # chunks 1-7
            for c in range(1, CT):
                ps = psp.tile([128, M], f32, tag="ps")
                eo = xp.tile([128, M], bf16)
                chunk_mms(ps, c, 0, MJ)
                nc.scalar.activation(eo[:], ps[:], AF.Exp, bias=ebias[:],
                                     scale=1.0, accum_out=dist_sb[:, c:c + 1])

            # psum partition q of chunk c is n = q*CT + c, so the "(p c)" view
            # writes dist in natural order
            nc.sync.dma_start(dist_d.ap().rearrange("(p c) -> p c", p=128), dist_sb[:])

    nc.compile()
    return nc


def _get_nc():
    if "nc" not in _CACHE:
        _CACHE["nc"] = _build_nc()
    return _CACHE["nc"]


def kernel(samples: np.ndarray, means: np.ndarray, stds: np.ndarray) -> np.ndarray:
    from concourse.bass_utils import run_bass_kernel_spmd

    samples = np.ascontiguousarray(samples, dtype=np.float32)
    means = np.ascontiguousarray(means, dtype=np.float32)
    stds = np.ascontiguousarray(stds, dtype=np.float32)

    nc = _get_nc()
    in_maps = [
        {"samples": samples[i * NSH:(i + 1) * NSH], "means": means, "stds": stds}
        for i in range(N_CORES)
    ]
    res = run_bass_kernel_spmd(nc, in_maps, list(range(N_CORES)))
    dist = np.concatenate([res.results[i]["dist"] for i in range(N_CORES)])
    return (-dist + dist.max() + dist.min()).astype(np.float32)
